# revision 1
# baseline (speedup 1.0000x reference)
"""Attention-pooling kernel for Trainium2 (Bass/Tile), 8-core data parallel.

Problem: for each batch item b (256 total):
    scores = E_b @ w_att            # [512]
    attn   = softmax(scores)        # [512]
    pooled = attn @ E_b             # [768]
    out_b  = sigmoid(pooled @ w_pred + b_pred)

Sharding: batch 256 -> 8 cores x 32 items. Weights replicated.

Per-core design (memory-bound; ~50 MiB of embeddings per core is the roofline):
  - E_b loaded once as [128, 4*768] f32 with s = 4p + c (12 KiB contiguous per
    partition -> clean DMA descriptors).
  - scores: one fused DVE tensor_tensor_reduce per s-chunk c:
        accum[p] = sum_d E[p,c,d] * w_att[d]   (w_att replicated to 128 parts)
  - u = exp(scores) on ScalarE (softmax max-subtraction skipped: scores ~ N(0,1),
    exp is safe in f32 and the math is identical).
  - pooled (and sum(u)) on PE: lhsT = u[:,c] (1-col stationary), rhs = E columns;
    out row goes to PSUM partition 32*(i%4) via tile_position col-groups, so 4
    items share one [128, 769] PSUM tile; accumulate over the 4 s-chunks.
    Column 768 (rhs = ones) accumulates U = sum(u).
  - finalize per 4-item group: one ScalarE PSUM->SBUF copy, one fused TTR against
    replicated w_pred, reciprocal of U, logits = dot/U + b_pred.
  - sigmoid once on the [128, 8] logit tile; 4 tiny DMAs write the [8,4] output.
"""

import os
import sys

import numpy as np

_REPO = "/opt/trn_rl_repo"
if _REPO not in sys.path:
    sys.path.insert(0, _REPO)

from contextlib import ExitStack

import concourse.bass as bass
import concourse.tile as tile
from concourse import bacc, mybir
from concourse.bass_utils import run_bass_kernel_spmd

N_CORES = 8
B = 256
S = 512
D = 768
PER_CORE = B // N_CORES  # 32
C = S // 128  # 4 s-chunks per item
GROUP = 4  # items per PSUM tile (col-groups 0/32/64/96)
WCAT = 2 * D + 1  # w_att | w_pred | b_pred

f32 = mybir.dt.float32
Alu = mybir.AluOpType
Act = mybir.ActivationFunctionType


def build_kernel(n_items: int = PER_CORE, group: int = GROUP):
    nc = bacc.Bacc(None, target_bir_lowering=False)

    emb = nc.dram_tensor("emb", [n_items, S, D], f32, kind="ExternalInput")
    wcat = nc.dram_tensor("wcat", [1, WCAT], f32, kind="ExternalInput")
    n_groups = (n_items + group - 1) // group
    out = nc.dram_tensor("out", [n_groups, group], f32, kind="ExternalOutput")

    with tile.TileContext(nc) as tc:
        with ExitStack() as ctx:
            const = ctx.enter_context(tc.tile_pool(name="const", bufs=1))
            e_pool = ctx.enter_context(tc.tile_pool(name="e", bufs=3))
            sc_pool = ctx.enter_context(tc.tile_pool(name="sc", bufs=4))
            scr_pool = ctx.enter_context(tc.tile_pool(name="scr", bufs=2))
            q_pool = ctx.enter_context(tc.tile_pool(name="q", bufs=2))
            fin_pool = ctx.enter_context(tc.tile_pool(name="fin", bufs=8))
            ps_q = ctx.enter_context(tc.tile_pool(name="psq", bufs=2, space="PSUM"))

            # ---- setup: replicate [w_att | w_pred | b_pred] to all 128 partitions
            wrep = const.tile([128, WCAT], f32)
            nc.gpsimd.dma_start(
                out=wrep[:, :], in_=wcat[0:1, :].broadcast_to([128, WCAT])
            )
            ones256 = const.tile([128, 256], f32)
            nc.vector.memset(ones256[:, :], 1.0)

            zall = const.tile([GROUP, n_groups], f32)

            psq = None
            for i in range(n_items):
                g, jj = divmod(i, group)
                et = e_pool.tile([128, C * D], f32, tag="et")
                src = emb[i : i + 1, :, :].rearrange(
                    "o (p c) d -> p (o c d)", p=128, c=C
                )
                nc.sync.dma_start(out=et[:, :], in_=src)

                sc = sc_pool.tile([128, C], f32, tag="sc")
                for c in range(C):
                    scr = scr_pool.tile([128, D], f32, tag="scr")
                    nc.vector.tensor_tensor(
                        out=scr[:, :],
                        in0=et[:, c * D : (c + 1) * D],
                        in1=wrep[:, 0:D],
                        op=Alu.mult,
                    )
                    scr2 = scr_pool.tile([128, D], f32, tag="scr2")
                    nc.scalar.activation(
                        out=scr2[:, :],
                        in_=scr[:, :],
                        func=Act.Copy,
                        accum_out=sc[:, c : c + 1],
                    )
                u16 = sc_pool.tile([128, C, group], f32, tag="u")
                nc.vector.memset(u16[:, :, :], 0.0)
                nc.scalar.activation(
                    out=u16[:, :, jj : jj + 1], in_=sc[:, :], func=Act.Exp
                )

                if jj == 0:
                    psq = ps_q.tile([group, 1024], f32, tag="psq")
                last_in_batch = jj == group - 1 or i == n_items - 1
                for lo, hi in ((0, 512), (512, 768), (768, 1024)):
                    for c in range(C):
                        rhs = (
                            ones256[:, :]
                            if lo == 768
                            else et[:, c * D + lo : c * D + hi]
                        )
                        # one accumulation group per PSUM bank per batch:
                        # bank0 = cols 0:512, bank1 = cols 512:1024 (two ranges)
                        nc.tensor.matmul(
                            out=psq[0:group, lo:hi],
                            lhsT=u16[:, c : c + 1, :],
                            rhs=rhs,
                            start=(jj == 0 and c == 0 and lo != 768),
                            stop=(last_in_batch and c == C - 1 and lo != 512),
                        )

                if last_in_batch:
                    qsb = q_pool.tile([group, D + 1], f32, tag="qsb")
                    nc.scalar.copy(out=qsb[:, :], in_=psq[0:group, 0 : D + 1])
                    dz = fin_pool.tile([group, 1], f32, tag="dz")
                    scrf = scr_pool.tile([group, D], f32, tag="scrf")
                    nc.vector.tensor_tensor(
                        out=scrf[:, :],
                        in0=qsb[:, 0:D],
                        in1=wrep[0:group, D : 2 * D],
                        op=Alu.mult,
                    )
                    scrf2 = scr_pool.tile([group, D], f32, tag="scrf2")
                    nc.scalar.activation(
                        out=scrf2[:, :],
                        in_=scrf[:, :],
                        func=Act.Copy,
                        accum_out=dz[:, :],
                    )
                    rU = fin_pool.tile([group, 1], f32, tag="rU")
                    nc.vector.reciprocal(out=rU[:, :], in_=qsb[:, D : D + 1])
                    t = fin_pool.tile([group, 1], f32, tag="t")
                    nc.vector.tensor_tensor(
                        out=t[:, :], in0=dz[:, :], in1=rU[:, :], op=Alu.mult
                    )
                    nc.vector.tensor_tensor(
                        out=zall[0:group, g : g + 1],
                        in0=t[:, :],
                        in1=wrep[0:group, 2 * D : 2 * D + 1],
                        op=Alu.add,
                    )

            sg = const.tile([GROUP, n_groups], f32)
            nc.scalar.activation(
                out=sg[0:group, :], in_=zall[0:group, :], func=Act.Sigmoid
            )
            nc.sync.dma_start(
                out=out[:, :].rearrange("g j -> j g"), in_=sg[0:group, 0:n_groups]
            )

    nc.compile()
    return nc


_NC_CACHE: dict[int, object] = {}


def _get_nc(n_items: int = PER_CORE):
    if n_items not in _NC_CACHE:
        _NC_CACHE[n_items] = build_kernel(n_items)
    return _NC_CACHE[n_items]


def make_runner(nc, in_maps):
    """Replicate bass2jax.run_bass_via_pjrt's multi-core path without output
    donation, returning (jitted_fn, device_args, out_names) so executions can
    be timed with inputs resident on device."""
    import jax
    import jax.numpy as jnp
    from jax.sharding import Mesh, PartitionSpec
    try:
        from jax.experimental.shard_map import shard_map
    except ImportError:
        from jax.shard_map import shard_map

    from concourse import bass2jax as b2j
    from concourse import mybir as mb

    b2j.install_neuronx_cc_hook()

    partition_name = nc.partition_id_tensor.name if nc.partition_id_tensor else None
    in_names, out_names, out_avals, zero_outs = [], [], [], []
    for alloc in nc.m.functions[0].allocations:
        if not isinstance(alloc, mb.MemoryLocationSet):
            continue
        name = alloc.memorylocations[0].name
        if alloc.kind == "ExternalInput":
            if name != partition_name:
                in_names.append(name)
        elif alloc.kind == "ExternalOutput":
            out_names.append(name)
            shape = tuple(alloc.tensor_shape)
            dtype = mb.dt.np(alloc.dtype)
            out_avals.append(jax.core.ShapedArray(shape, dtype))
            zero_outs.append(np.zeros(shape, dtype))
    n_params = len(in_names)
    all_in_names = list(in_names) + list(out_names)
    if partition_name is not None:
        all_in_names.append(partition_name)

    def _body(*args):
        operands = list(args)
        if partition_name is not None:
            operands.append(b2j.partition_id_tensor())
        outs = b2j._bass_exec_p.bind(
            *operands,
            out_avals=tuple(out_avals),
            in_names=tuple(all_in_names),
            out_names=tuple(out_names),
            lowering_input_output_aliases=(),
            sim_require_finite=True,
            sim_require_nnan=True,
            nc=nc,
        )
        return tuple(outs)

    n_cores = len(in_maps)
    devices = jax.devices()[:n_cores]
    mesh = Mesh(np.asarray(devices), ("core",))
    in_specs = (PartitionSpec("core"),) * (n_params + len(out_names))
    out_specs = (PartitionSpec("core"),) * len(out_names)
    fn = jax.jit(
        shard_map(
            _body, mesh=mesh, in_specs=in_specs, out_specs=out_specs, check_rep=False
        ),
        keep_unused=True,
    )
    per_core = [[np.asarray(m[name]) for name in in_names] for m in in_maps]
    concat_in = [
        np.concatenate([per_core[c][i] for c in range(n_cores)], axis=0)
        for i in range(n_params)
    ]
    concat_zeros = [
        np.zeros((n_cores * z.shape[0], *z.shape[1:]), z.dtype) for z in zero_outs
    ]
    sharding = jax.sharding.NamedSharding(mesh, PartitionSpec("core"))
    args = [jax.device_put(a, sharding) for a in concat_in + concat_zeros]
    return fn, args, out_names, out_avals


def kernel(embeddings, w_att, w_pred, b_pred, **run_kwargs):
    embeddings = np.ascontiguousarray(embeddings, dtype=np.float32)
    w_att = np.asarray(w_att, dtype=np.float32).reshape(D)
    w_pred = np.asarray(w_pred, dtype=np.float32).reshape(D)
    b_pred = np.float32(np.asarray(b_pred).reshape(()))
    wcat = np.concatenate([w_att, w_pred, [b_pred]]).astype(np.float32)
    wcat = wcat.reshape(1, WCAT)

    nc = _get_nc(PER_CORE)
    in_maps = [
        {
            "emb": embeddings[i * PER_CORE : (i + 1) * PER_CORE],
            "wcat": wcat,
        }
        for i in range(N_CORES)
    ]
    res = run_bass_kernel_spmd(nc, in_maps, core_ids=list(range(N_CORES)), **run_kwargs)
    outs = [res.results[i]["out"].reshape(-1)[:PER_CORE] for i in range(N_CORES)]
    full = np.concatenate(outs).astype(np.float32)
    if run_kwargs:
        return full, res
    return full

